# revision 12
# baseline (speedup 1.0000x reference)
"""Trainium2 Bass kernel for the Lorentz (hyperboloid) embedding loss.

Data-parallel over the batch: B=16384 anchors are sharded 2048-per-core
across 8 NeuronCores. Per anchor the kernel needs the anchor row plus its
50 candidate rows of the 1M x 32 fp32 table. The embedding-row
indirection is resolved on the host into a densely packed per-core
operand (the container's compile path mis-lowers every indirect/gather
DMA primitive - see notes); each core then streams its 13.4MB of rows at
HBM line rate and computes the Lorentz scalar products (VectorE),
arcosh (ScalarE), and the logsumexp loss (ScalarE+VectorE) fully
on-device, writing its 2048 losses. The host concatenates the 8 shards.
"""
import os
import sys

for _p in ("/opt/trn_rl_repo", "/root/.axon_site/_ro/trn_rl_repo"):
    if _p not in sys.path and os.path.isdir(_p):
        sys.path.append(_p)

import numpy as np

N_ITEMS_P1 = 1_000_001
DIM = 32
B = 16384
N_KS = 50
W = N_KS + 1          # rows per anchor: anchor + 50 candidates
P = 128               # SBUF partitions = anchors per tile
N_CORES = 8
B_SHARD = B // N_CORES
N_TILES = B_SHARD // P

_CLAMP = float(np.float32(1.0 + 1e-6))

_nc_cache = None


def _build():
    import concourse.bacc as bacc
    import concourse.tile as tile
    from concourse import mybir

    F32 = mybir.dt.float32
    AF = mybir.ActivationFunctionType
    OP = mybir.AluOpType

    nc = bacc.Bacc(
        "TRN2", target_bir_lowering=False, debug=False, num_devices=N_CORES
    )
    # g[b, 0, :] = table[I[b]]; g[b, 1+n, :] = table[Ks[b, n]]  (host-packed)
    g_in = nc.declare_dram_parameter("g", [B_SHARD, W * DIM], F32, isOutput=False)
    loss = nc.declare_dram_parameter("loss", [B_SHARD], F32, isOutput=True)

    with tile.TileContext(nc) as tc:
        with (
            tc.tile_pool(name="cons", bufs=1) as cons,
            tc.tile_pool(name="big", bufs=6) as big,
            tc.tile_pool(name="small", bufs=2) as small,
        ):
            bias_neg1 = cons.tile([P, 1], F32)
            nc.vector.memset(bias_neg1[:], -1.0)
            bias_eps = cons.tile([P, 1], F32)
            nc.vector.memset(bias_eps[:], 1e-6)

            GRP = 4                      # tiles batched per activation group
            GW = GRP * N_KS              # 200
            TPI = 2                      # tiles fused per DVE instruction
            for gi in range(N_TILES // GRP):
                dB = small.tile([P, GRP, N_KS], F32, tag="dB")
                for tg in range(0, GRP, TPI):
                    t = gi * GRP + tg
                    # partition p holds rows of anchors t*128+p and (t+1)*128+p
                    g = big.tile([P, TPI, W * DIM], F32, tag="g")
                    src = g_in[t * P:(t + TPI) * P, :].rearrange(
                        "(c p) f -> p c f", p=P
                    )
                    nc.sync.dma_start(out=g[:], in_=src)
                    # m[p, c, n, d] = ui[p, c, d] * uks[p, c, n, d]
                    m = big.tile([P, TPI, N_KS, DIM], F32, tag="m")
                    g4 = g[:].rearrange("p c (w d) -> p c w d", d=DIM)
                    nc.vector.tensor_tensor(
                        out=m[:],
                        in0=g4[:, :, 1:, :],
                        in1=g4[:, :, 0:1, :].to_broadcast([P, TPI, N_KS, DIM]),
                        op=OP.mult,
                    )
                    # d = -lsp = m[...,0] - sum_{d>=1} m (same op order as reference)
                    ssp = small.tile([P, TPI, N_KS], F32, tag="ssp")
                    nc.vector.tensor_reduce(
                        out=ssp[:], in_=m[:, :, :, 1:],
                        axis=mybir.AxisListType.X, op=OP.add,
                    )
                    nc.vector.tensor_tensor(
                        out=dB[:, tg:tg + TPI, :],
                        in0=m[:, :, :, 0],
                        in1=ssp[:],
                        op=OP.subtract,
                    )
                # exact select over the whole group: d' = d if d > 1 else 1+1e-6
                dF = dB[:].opt()                       # [P, 200] view
                mask = small.tile([P, GW], F32, tag="mask")
                nc.vector.tensor_scalar(
                    out=mask[:], in0=dF, scalar1=1.0, scalar2=None, op0=OP.is_gt
                )
                dm = small.tile([P, GW], F32, tag="dm")
                nc.vector.tensor_scalar(
                    out=dm[:], in0=dF, scalar1=_CLAMP, scalar2=None, op0=OP.subtract
                )
                nc.vector.tensor_tensor(out=dm[:], in0=dm[:], in1=mask[:], op=OP.mult)
                nc.vector.tensor_scalar(
                    out=dm[:], in0=dm[:], scalar1=_CLAMP, scalar2=None, op0=OP.add
                )
                # t = d' + sqrt(d'*d' - 1);  arcosh(d') = ln(t)
                q = small.tile([P, GW], F32, tag="q")
                nc.vector.tensor_tensor(out=q[:], in0=dm[:], in1=dm[:], op=OP.mult)
                r = small.tile([P, GW], F32, tag="r")
                nc.scalar.activation(out=r[:], in_=q[:], func=AF.Sqrt, bias=bias_neg1[:])
                tt = small.tile([P, GRP, N_KS], F32, tag="tt")
                nc.vector.tensor_tensor(
                    out=tt[:].opt(), in0=dm[:], in1=r[:], op=OP.add
                )
                # a0 = ln(t) for the positive column of each tile in the group
                a0 = small.tile([P, GRP], F32, tag="a0")
                nc.scalar.activation(out=a0[:], in_=tt[:, :, 0], func=AF.Ln)
                # exp(-arcosh(d')) = exp(-ln(t)) = 1/t ; sum over the 50 slots
                rec = small.tile([P, GRP, N_KS], F32, tag="rec")
                nc.vector.reciprocal_approx_fast(out=rec[:].opt(), in_=tt[:].opt())
                s1 = small.tile([P, GRP], F32, tag="s1")
                nc.vector.tensor_reduce(
                    out=s1[:], in_=rec[:], axis=mybir.AxisListType.X, op=OP.add
                )
                lse = small.tile([P, GRP], F32, tag="lse")
                nc.scalar.activation(out=lse[:], in_=s1[:], func=AF.Ln, bias=bias_eps[:])
                # loss = a0 + lse
                lv = small.tile([P, GRP], F32, tag="lv")
                nc.vector.tensor_tensor(out=lv[:], in0=a0[:], in1=lse[:], op=OP.add)
                for tg in range(GRP):
                    t = gi * GRP + tg
                    nc.gpsimd.dma_start(
                        out=loss[t * P:(t + 1) * P, None], in_=lv[:, tg:tg + 1]
                    )
    nc.compile()
    return nc


def _get_nc():
    global _nc_cache
    if _nc_cache is None:
        _nc_cache = _build()
    return _nc_cache


def _prep_in_maps(table, I, Ks):
    table = np.ascontiguousarray(np.asarray(table, dtype=np.float32))
    I = np.asarray(I).astype(np.int64)
    Ks = np.asarray(Ks).astype(np.int64)
    assert table.shape == (N_ITEMS_P1, DIM)
    assert I.shape == (B,) and Ks.shape == (B, N_KS)
    ik = np.concatenate([I[:, None], Ks], axis=1)       # [B, 51]
    g_full = table[ik.reshape(-1)].reshape(B, W * DIM)  # [B, 51*32]
    in_maps = []
    for c in range(N_CORES):
        sh = np.ascontiguousarray(g_full[c * B_SHARD:(c + 1) * B_SHARD])
        in_maps.append({"g": sh})
    return in_maps


def _run(table, I, Ks, trace=False, **kwargs):
    from concourse.bass_utils import run_bass_kernel_spmd

    nc = _get_nc()
    in_maps = _prep_in_maps(table, I, Ks)
    res = run_bass_kernel_spmd(
        nc, in_maps, list(range(N_CORES)), trace=trace, **kwargs
    )
    out = np.concatenate(
        [np.asarray(res.results[c]["loss"]) for c in range(N_CORES)]
    ).astype(np.float32)
    return out, res


def kernel(table, I, Ks):
    out, _ = _run(table, I, Ks, trace=False)
    return out
